# revision 1
# baseline (speedup 1.0000x reference)
"""Block-sparse attention (CABAttention) Trainium2 kernel — v3.

Sharding: 8 cores = 2 batches x 4 head-groups (4 heads each).
Per core: qkv projection (fp16 in, fp32 PSUM), top-2+diag block-sparse
attention (fp16 value path), output projection (row-parallel partial
sums, f16, host-summed + bias).

v3: PE-dynamic architecture (dynamic block offsets consumed as
register-offset moving operands on the tensor engine — the only
stable high-rate dynamic path: SP-issued dynamic DMAs cost ~730ns
sequencer time each, and DVE/ACT reg_loads hang the device). vs the
v1 baseline: ONE 4-register load per (pair, qblock) iteration instead
of four serialized single loads, fp16 qkv projection, anchor distance
6 (not 3), and the output projection interleaved into the attention
loop so the PE stays dense and the output DMA overlaps compute.
Block selection (top-2 of coarse block-mean scores) runs on host in
float64 and is passed as index inputs.
"""
import sys

sys.path.insert(0, "/opt/trn_rl_repo")

import numpy as np

import concourse.bass as bass
import concourse.mybir as mybir
import concourse.tile as tile
from concourse import bacc
from concourse.bass import ds
from concourse.bass_utils import run_bass_kernel_spmd
from concourse.masks import make_identity

F32 = mybir.dt.float32
F16 = mybir.dt.float16
I32 = mybir.dt.int32
ET = mybir.EngineType

DIM = 1024
H = 16
HD = 64
BS = 64
N = 2048
B = 2
M = N // BS            # 32 blocks
SCALE = HD ** -0.5
NCORES = 8
HPC = H // (NCORES // B)   # 4 heads per core

_NC_CACHE = None
LAST_RESULTS = None


def build_kernel():
    import os
    anchor_d = int(os.environ.get("ANCHOR_D", "8"))
    nc = bacc.Bacc(None)
    xt_d = nc.dram_tensor("xt", [DIM, N], F16, kind="ExternalInput")
    wq_d = nc.dram_tensor("wq", [DIM, 768], F16, kind="ExternalInput")
    pw_d = nc.dram_tensor("pw", [256, DIM], F16, kind="ExternalInput")
    idx_d = nc.dram_tensor("selidx", [1, 256], I32, kind="ExternalInput")
    wb_d = nc.dram_tensor("wbias", [128, 64], F32, kind="ExternalInput")
    y_d = nc.dram_tensor("y", [N, DIM], F16, kind="ExternalOutput")

    with tile.TileContext(nc) as tc:
        with tc.tile_pool(name="big", bufs=1) as big, \
             tc.tile_pool(name="wrk", bufs=8) as wrk:

            # ---- persistent SBUF tensors ----
            xt = big.tile([128, 8, N], F16)           # x^T, feature-major
            wq = big.tile([128, 8, 768], F16)         # qkv weights^T
            pwt = big.tile([128, 2, DIM], F16)        # proj weights
            idx = big.tile([1, 256], I32)
            wb = big.tile([128, 64], F32)
            qT = [big.tile([128, N], F16, name=f"qT{i}") for i in range(2)]
            kkT = [big.tile([128, N], F16, name=f"kkT{i}") for i in range(2)]
            vdAB = [big.tile([64, 2, N], F16, name=f"vdAB{i}")
                    for i in range(2)]
            vdA = [vdAB[i][:, 0, :] for i in range(2)]
            vdB = [vdAB[i][:, 1, :] for i in range(2)]
            outT = [big.tile([128, N], F16, name=f"outT{i}") for i in range(2)]
            qTB = [big.tile([64, N], F16, name=f"qTB{i}") for i in range(2)]
            kkTB = [big.tile([64, N], F16, name=f"kkTB{i}") for i in range(2)]
            identf = big.tile([128, 128], F32)
            ident = big.tile([128, 128], F16)

            # ---- input DMAs (split for pipelining) ----
            xt_v = xt_d[:].rearrange("(a p) n -> p a n", p=128)
            wq_v = wq_d[:].rearrange("(a p) n -> p a n", p=128)
            pw_v = pw_d[:].rearrange("(a p) n -> p a n", p=128)
            for k in range(8):
                nc.sync.dma_start(xt[:, k, :], xt_v[:, k, :])
                nc.sync.dma_start(wq[:, k, :], wq_v[:, k, :])
            nc.sync.dma_start(pwt[:], pw_v[:])
            idx_dma = nc.sync.dma_start(idx[:], idx_d[:])
            nc.sync.dma_start(wb[:], wb_d[:])

            make_identity(nc, identf[:])
            nc.vector.tensor_copy(ident[:], identf[:])

            # ---- qkv projection: fp16 inputs, fp32 PSUM over 8 K-chunks ----
            # K first (attention needs it first), then V token-major
            # (stationary = x token-tiles: lands directly in vd layout,
            # no transpose pass), Q last; head-B base-0 dup DMAs issue
            # as soon as their source is complete.
            # wq columns: q0 0:128 | q1 128:256 | k0 256:384 |
            #             k1 384:512 | v (p0 hA|hB, p1 hA|hB) 512:768
            tgt = {0: qT[0], 1: qT[1], 2: kkT[0], 3: kkT[1]}
            with tc.tile_pool(name="qkps", bufs=6, space="PSUM") as qkps, \
                 tc.tile_pool(name="vqps", bufs=2, space="PSUM") as vqps:

                def mtile(mt):
                    for nt in range(4):
                        ps = qkps.tile([128, 512], F32)
                        for k in range(8):
                            nc.tensor.matmul(
                                ps[:], lhsT=wq[:, k, mt * 128:(mt + 1) * 128],
                                rhs=xt[:, k, nt * 512:(nt + 1) * 512],
                                start=(k == 0), stop=(k == 7))
                        nc.vector.tensor_copy(
                            tgt[mt][:, nt * 512:(nt + 1) * 512], ps[:])

                def vtile(tt):
                    ps = vqps.tile([128, 256], F32, tag="vps")
                    for k in range(8):
                        nc.tensor.matmul(
                            ps[:], lhsT=xt[:, k, tt * 128:(tt + 1) * 128],
                            rhs=wq[:, k, 512:768],
                            start=(k == 0), stop=(k == 7))
                    for half in range(2):
                        j = 2 * tt + half
                        rs = slice(half * 64, (half + 1) * 64)
                        js = slice(j * 64, (j + 1) * 64)
                        for p in range(2):
                            for h in range(2):
                                c0 = (p * 2 + h) * 64
                                nc.vector.tensor_copy(
                                    vdAB[p][:, h, js],
                                    ps[rs, c0:c0 + 64])

                mtile(2)
                nc.sync.dma_start(kkTB[0][:], kkT[0][64:128, :])
                mtile(3)
                nc.sync.dma_start(kkTB[1][:], kkT[1][64:128, :])
                for tt in range(16):
                    vtile(tt)
                mtile(0)
                nc.sync.dma_start(qTB[0][:], qT[0][64:128, :])
                mtile(1)
                nc.sync.dma_start(qTB[1][:], qT[1][64:128, :])

            # ---- block-sparse attention + interleaved projection ----
            # software-pipelined: stage A (scores+softmax), B (transpose
            # probs), C (AV), D (out transpose) of four consecutive
            # iterations interleave so the PE never waits on the
            # DVE/ACT softmax round-trips.
            with tc.tile_pool(name="fsps", bufs=4, space="PSUM") as fsps, \
                 tc.tile_pool(name="hsps", bufs=3, space="PSUM") as hsps, \
                 tc.tile_pool(name="ypsp", bufs=1, space="PSUM") as ypsp:
                anchors = {}
                st = {}
                NT = 2 * M

                def stage_a(t):
                    qb, p = t // 2, t % 2
                    qs = slice(qb * 64, (qb + 1) * 64)
                    base = p * 128 + qb * 4
                    loads, offs = nc.values_load_multi_w_load_instructions(
                        idx[0:1, base:base + 4], engines=[ET.PE],
                        min_val=0, max_val=N - 64,
                        skip_runtime_bounds_check=True)
                    for li in loads:
                        tile.add_dep_helper(li.ins, idx_dma.ins, sync=True,
                                            reason="idx dma -> pe regs")
                        if t >= anchor_d:
                            tile.add_dep_helper(
                                li.ins, anchors[t - anchor_d].ins, sync=False,
                                reason="bound PE register live range")
                    oA1, oA2, oB1, oB2 = offs
                    fs = fsps.tile([128, 320], F32, tag="fs")
                    sps = fs[:, 0:192]
                    for s, (oa, ob) in enumerate(
                            [(oA1, oB1), (oA2, oB2),
                             (qb * 64, qb * 64)]):
                        cs = slice(s * 64, (s + 1) * 64)
                        nc.tensor.matmul(
                            sps[0:64, cs], lhsT=qT[p][0:64, qs],
                            rhs=kkT[p][0:64, ds(oa, 64)],
                            start=True, stop=True)
                        nc.tensor.matmul(
                            sps[64:128, cs], lhsT=qTB[p][:, qs],
                            rhs=kkTB[p][:, ds(ob, 64)],
                            start=True, stop=True,
                            skip_group_check=True,
                            tile_position=(0, 64))
                    # mask duplicated diag slot (bias -30000 -> exp 0)
                    nc.scalar.activation(
                        sps[:, 128:192], sps[:, 128:192],
                        mybir.ActivationFunctionType.Identity,
                        bias=wb[:, p * 32 + qb:p * 32 + qb + 1])
                    pu = wrk.tile([128, 192], F16, tag="pu")
                    den = wrk.tile([128, 1], F32, tag="den")
                    nc.scalar.activation(pu[:], sps[:],
                                         mybir.ActivationFunctionType.Exp,
                                         accum_out=den[:])
                    rden = wrk.tile([128, 1], F32, tag="rden")
                    nc.vector.reciprocal(rden[:], den[:])
                    pr = wrk.tile([128, 192], F16, tag="pr")
                    nc.vector.tensor_scalar(pr[:], pu[:], rden[:, 0:1], None,
                                            op0=mybir.AluOpType.mult)
                    st[t] = {"offs": offs, "fs": fs, "pr": pr}

                def stage_b(t):
                    s_ = st[t]
                    hs = hsps.tile([128, 448], F16, tag="hs")
                    pt = hs[0:64, 0:384]
                    for s in range(3):
                        nc.tensor.transpose(
                            pt[:, s * 128:(s + 1) * 128],
                            s_["pr"][:, s * 64:(s + 1) * 64], ident[:])
                    pts = wrk.tile([64, 384], F16, tag="pts")
                    nc.vector.tensor_copy(pts[:], pt[:])
                    s_["hs"] = hs
                    s_["pts"] = pts

                def stage_c(t):
                    qb, p = t // 2, t % 2
                    s_ = st[t]
                    oA1, oA2, oB1, oB2 = s_["offs"]
                    pts = s_["pts"]
                    avpAB = s_["fs"][0:64, 192:320]
                    for s, o in enumerate([oA1, oA2, qb * 64]):
                        nc.tensor.matmul(
                            avpAB[:, 0:64],
                            lhsT=pts[:, s * 128:s * 128 + 64],
                            rhs=vdA[p][:, ds(o, 64)],
                            start=(s == 0), stop=(s == 2))
                    for s, o in enumerate([oB1, oB2, qb * 64]):
                        mi = nc.tensor.matmul(
                            avpAB[:, 64:128],
                            lhsT=pts[:, s * 128 + 64:s * 128 + 128],
                            rhs=vdB[p][:, ds(o, 64)],
                            start=(s == 0), stop=(s == 2))
                    anchors[t] = mi
                    av_sb = wrk.tile([64, 128], F16, tag="av_sb")
                    nc.scalar.copy(av_sb[:], avpAB[:])
                    s_["av_sb"] = av_sb

                def stage_d(t):
                    qb, p = t // 2, t % 2
                    qs = slice(qb * 64, (qb + 1) * 64)
                    s_ = st.pop(t)
                    otp = s_["hs"][:, 384:448]
                    nc.tensor.transpose(otp[:], s_["av_sb"][:],
                                        ident[0:64, 0:64])
                    nc.vector.tensor_copy(outT[p][:, qs], otp[:])

                def proj_half(tt, nt):
                    ts_ = slice(tt * 128, (tt + 1) * 128)
                    ns = slice(nt * 512, (nt + 1) * 512)
                    yp = ypsp.tile([128, 512], F32)
                    nc.tensor.matmul(yp[:], lhsT=outT[0][:, ts_],
                                     rhs=pwt[:, 0, ns],
                                     start=True, stop=False)
                    nc.tensor.matmul(yp[:], lhsT=outT[1][:, ts_],
                                     rhs=pwt[:, 1, ns],
                                     start=False, stop=True)
                    ys = wrk.tile([128, 512], F16, tag="ys")
                    nc.vector.tensor_copy(ys[:], yp[:])
                    nc.sync.dma_start(y_d[ts_, ns], ys[:])

                # iteration order: p inner-fast would split pairs; use
                # t = qb*2 + p so both pairs of a token tile finish on
                # consecutive t (proj after t = 4*tt+3 completes stage D)
                for w in range(NT + 3):
                    if w < NT:
                        stage_a(w)
                    if 0 <= w - 1 < NT:
                        stage_b(w - 1)
                    if 0 <= w - 2 < NT:
                        stage_c(w - 2)
                    if 0 <= w - 3 < NT:
                        td = w - 3
                        stage_d(td)
                        if td % 4 == 3:
                            proj_half(td // 4, 0)
                        elif td % 4 == 1 and td >= 5:
                            proj_half((td - 5) // 4, 1)
                proj_half(M // 2 - 1, 1)

    nc.finalize()
    return nc


def _host_prep(x, qkv_w, proj_w):
    """Per-core input maps + block selection (float64, matches fp32 ref)."""
    in_maps = []
    x64 = x.astype(np.float64)
    for core in range(NCORES):
        b = core // (NCORES // B)
        hg = core % (NCORES // B)
        heads = [hg * HPC + i for i in range(HPC)]

        xt = np.ascontiguousarray(x[b].T).astype(np.float16)

        wqkvT = np.empty((DIM, 768), np.float32)
        for p in range(2):
            hA, hB = heads[2 * p], heads[2 * p + 1]
            wqkvT[:, p*128:p*128+64] = qkv_w[hA*64:(hA+1)*64].T * SCALE
            wqkvT[:, p*128+64:p*128+128] = qkv_w[hB*64:(hB+1)*64].T * SCALE
            kbase = 256 + p * 128
            wqkvT[:, kbase:kbase+64] = qkv_w[DIM+hA*64:DIM+(hA+1)*64].T
            wqkvT[:, kbase+64:kbase+128] = qkv_w[DIM+hB*64:DIM+(hB+1)*64].T
            vbase = 512 + p * 128
            wqkvT[:, vbase:vbase+64] = qkv_w[2*DIM+hA*64:2*DIM+(hA+1)*64].T
            wqkvT[:, vbase+64:vbase+128] = qkv_w[2*DIM+hB*64:2*DIM+(hB+1)*64].T

        pw = np.ascontiguousarray(
            proj_w[:, heads[0]*64:(heads[-1]+1)*64].T).astype(np.float16)

        # float64 selection (matches fp32 reference ordering w/ margin)
        xb = x64[b].reshape(M, BS, DIM).mean(axis=1)
        selidx = np.zeros((1, 256), np.int32)
        wbias = np.zeros((128, 64), np.float32)
        for p in range(2):
            for hip in range(2):
                h = heads[2 * p + hip]
                qb_ = xb @ qkv_w[h*64:(h+1)*64].T.astype(np.float64)
                kb_ = xb @ qkv_w[DIM+h*64:DIM+(h+1)*64].T.astype(np.float64)
                c = qb_ @ kb_.T
                for i in range(M):
                    order = np.argsort(-c[i], kind="stable")
                    i1, i2 = int(order[0]), int(order[1])
                    col = p * 128 + i * 4 + hip * 2
                    selidx[0, col] = i1 * 64
                    selidx[0, col + 1] = i2 * 64
                    if i == i1 or i == i2:
                        wbias[hip*64:(hip+1)*64, p*32+i] = -30000.0
        in_maps.append({"xt": xt, "wq": wqkvT.astype(np.float16), "pw": pw,
                        "selidx": selidx, "wbias": wbias})
    return in_maps


def kernel(x, qkv_w, proj_w, proj_b):
    global _NC_CACHE, LAST_RESULTS
    x = np.asarray(x, np.float32)
    qkv_w = np.asarray(qkv_w, np.float32)
    proj_w = np.asarray(proj_w, np.float32)
    proj_b = np.asarray(proj_b, np.float32)

    if _NC_CACHE is None:
        _NC_CACHE = build_kernel()
    nc = _NC_CACHE

    in_maps = _host_prep(x, qkv_w, proj_w)
    res = run_bass_kernel_spmd(nc, in_maps, list(range(NCORES)))
    LAST_RESULTS = res

    out = np.zeros((B, N, DIM), np.float32)
    for core in range(NCORES):
        out[core // (NCORES // B)] += res.results[core]["y"].astype(np.float32)
    out += proj_b[None, None, :]
    return out



# revision 6
# speedup vs baseline: 1.0193x; 1.0193x over previous
"""Block-sparse attention (CABAttention) Trainium2 kernel — v4.

Sharding: 8 cores = 2 batches x 4 head-groups (4 heads each).
Per core: qkv projection (fp16 in, fp32 PSUM), top-2+diag block-sparse
attention, output projection (row-parallel partial sums, host-summed).

v4 vs v3: the dynamic block selection moves OFF the tensor engine.
GPSIMD ap_gather pre-gathers the selected K blocks (with diag; one
192-wide score matmul per head-half) and V blocks (without diag) into
packed SBUF buffers using host-provided wrapped index tensors, so every
attention matmul is statically addressed — no PE register loads, no
per-iteration ucode ops. The dup-diag mask lands via a 1-contraction
outer-product matmul accumulated into PSUM instead of an ACT pass, and
softmax normalization happens after AV (scale at PSUM evacuation) so the
probability tensor is never rescaled on the DVE. Phase order keeps the
PE dense for the HAM clock gate: K-projection streams k-outer against
the input DMA, Q1-projection fills attention p=0, output projection
fills attention p=1.
"""
import sys

sys.path.insert(0, "/opt/trn_rl_repo")

import numpy as np

import concourse.bass as bass
import concourse.mybir as mybir
import concourse.tile as tile
from concourse import bacc
from concourse.bass_utils import run_bass_kernel_spmd
from concourse.masks import make_identity

F32 = mybir.dt.float32
F16 = mybir.dt.float16
I16 = mybir.dt.int16
ET = mybir.EngineType

DIM = 1024
H = 16
HD = 64
BS = 64
N = 2048
B = 2
M = N // BS            # 32 blocks
SCALE = HD ** -0.5
NCORES = 8
HPC = H // (NCORES // B)   # 4 heads per core

_NC_CACHE = None
LAST_RESULTS = None


def build_kernel():
    nc = bacc.Bacc(None)
    xt_d = nc.dram_tensor("xt", [DIM, N], F16, kind="ExternalInput")
    wq_d = nc.dram_tensor("wq", [DIM, 768], F16, kind="ExternalInput")
    pw_d = nc.dram_tensor("pw", [256, DIM], F16, kind="ExternalInput")
    idx_d = nc.dram_tensor("selidx", [128, 28], I16, kind="ExternalInput")
    wbm_d = nc.dram_tensor("wbmask", [1, 64 * 128], F16, kind="ExternalInput")
    y_d = nc.dram_tensor("y", [N, DIM], F16, kind="ExternalOutput")

    with tile.TileContext(nc) as tc:
        with tc.tile_pool(name="big", bufs=1) as big, \
             tc.tile_pool(name="wrk", bufs=8) as wrk:

            # ---- persistent SBUF tensors ----
            xt = big.tile([128, 8, N], F16)           # x^T, feature-major
            wq = big.tile([128, 8, 768], F16)         # qkv weights^T
            pwt = big.tile([128, 2, DIM], F16)        # proj weights
            idx = big.tile([128, 28], I16)
            wbm = big.tile([1, 64, 128], F16)
            ones = big.tile([1, 64], F16)
            qT = [big.tile([128, N], F16, name=f"qT{i}") for i in range(2)]
            kkT = [big.tile([128, N], F16, name=f"kkT{i}") for i in range(2)]
            vdAB = [big.tile([64, 2, N], F16, name=f"vdAB{i}")
                    for i in range(2)]
            ksel = [big.tile([128, 3 * M, 64], F16, name=f"ksel{i}")
                    for i in range(2)]
            vsel = [big.tile([64, 2, 2 * M, 64], F16, name=f"vsel{i}")
                    for i in range(2)]
            outT = [big.tile([128, N], F16, name=f"outT{i}") for i in range(2)]
            identf = big.tile([128, 128], F32)
            ident = big.tile([128, 128], F16)

            # ---- input DMAs (chunked so K-proj streams behind them) ----
            xt_v = xt_d[:].rearrange("(a p) n -> p a n", p=128)
            wq_v = wq_d[:].rearrange("(a p) n -> p a n", p=128)
            pw_v = pw_d[:].rearrange("(a p) n -> p a n", p=128)
            for k in range(8):
                nc.sync.dma_start(wq[:, k, :], wq_v[:, k, :])
                nc.sync.dma_start(xt[:, k, :], xt_v[:, k, :])
            nc.sync.dma_start(pwt[:], pw_v[:])
            nc.sync.dma_start(idx[:], idx_d[:])
            nc.sync.dma_start(wbm[:], wbm_d[:].rearrange("o (c p) -> o c p",
                                                         c=64))

            make_identity(nc, identf[:])
            nc.vector.tensor_copy(ident[:], identf[:])
            nc.gpsimd.memset(ones[:], 1.0)

            # ---- K projection: k-outer, streams behind the input DMA ----
            # wq columns: q0 0:128 | q1 128:256 | k0 256:384 | k1 384:512 |
            #             v (p0 hA|hB, p1 hA|hB) 512:768
            with tc.tile_pool(name="kps", bufs=1, space="PSUM") as kps:
                kp = [[kps.tile([128, 512], F32, name=f"kp{p}{nt}")
                       for nt in range(4)] for p in range(2)]
                for k in range(8):
                    for p in range(2):
                        for nt in range(4):
                            nc.tensor.matmul(
                                kp[p][nt][:],
                                lhsT=wq[:, k, 256 + p * 128:384 + p * 128],
                                rhs=xt[:, k, nt * 512:(nt + 1) * 512],
                                start=(k == 0), stop=(k == 7))
                # evacuations spread across engines
                for p in range(2):
                    for nt in range(4):
                        if nt % 2:
                            nc.scalar.copy(
                                kkT[p][:, nt * 512:(nt + 1) * 512],
                                kp[p][nt][:])
                        else:
                            nc.vector.tensor_copy(
                                kkT[p][:, nt * 512:(nt + 1) * 512],
                                kp[p][nt][:])

            # ---- K gathers (gpsimd; 3 blocks/qblock incl diag) ----
            for p in range(2):
                nc.gpsimd.ap_gather(
                    ksel[p][:].rearrange("p m d -> p (m d)"),
                    kkT[p][:],
                    idx[:, p * 6:(p + 1) * 6],
                    channels=128, num_elems=M, d=64, num_idxs=3 * M)

            # ---- V projection (dense) ----
            with tc.tile_pool(name="vps", bufs=2, space="PSUM") as vps:
                for tt in range(16):
                    ps = vps.tile([128, 256], F32, tag="vps")
                    for k in range(8):
                        nc.tensor.matmul(
                            ps[:], lhsT=xt[:, k, tt * 128:(tt + 1) * 128],
                            rhs=wq[:, k, 512:768],
                            start=(k == 0), stop=(k == 7))
                    for half in range(2):
                        j = 2 * tt + half
                        rs = slice(half * 64, (half + 1) * 64)
                        js = slice(j * 64, (j + 1) * 64)
                        for p in range(2):
                            for h in range(2):
                                c0 = (p * 2 + h) * 64
                                nc.vector.tensor_copy(
                                    vdAB[p][:, h, js],
                                    ps[rs, c0:c0 + 64])

            # ---- V gathers (gpsimd; 2 blocks/qblock, diag stays static) ----
            for p in range(2):
                for h in range(2):
                    nc.gpsimd.ap_gather(
                        vsel[p][:, h].rearrange("p m d -> p (m d)"),
                        vdAB[p][:, h, :],
                        idx[0:64, 12 + (p * 2 + h) * 4:16 + (p * 2 + h) * 4],
                        channels=64, num_elems=M, d=64, num_idxs=2 * M)

            # ---- Q0 projection (dense) ----
            # qq pool (2 banks) is shared by Q0, Q1 and the output
            # projection — they are temporally disjoint.
            with tc.tile_pool(name="qq", bufs=2, space="PSUM") as qq, \
                 tc.tile_pool(name="fsps", bufs=3, space="PSUM") as fsps, \
                 tc.tile_pool(name="hsps", bufs=3, space="PSUM") as hsps:

                for nt in range(4):
                    ps = qq.tile([128, 512], F32, tag="q")
                    for k in range(8):
                        nc.tensor.matmul(
                            ps[:], lhsT=wq[:, k, 0:128],
                            rhs=xt[:, k, nt * 512:(nt + 1) * 512],
                            start=(k == 0), stop=(k == 7))
                    nc.vector.tensor_copy(
                        qT[0][:, nt * 512:(nt + 1) * 512], ps[:])

                # ---- attention (p-major) + interleaved Q1 / projection ----
                st = {}
                NT = 2 * M
                q1tile = [None]

                def q1_mm(i):
                    nt, k = i // 8, i % 8
                    if k == 0:
                        q1tile[0] = qq.tile([128, 512], F32, tag="q",
                                            name="q1t")
                    nc.tensor.matmul(
                        q1tile[0][:], lhsT=wq[:, k, 128:256],
                        rhs=xt[:, k, nt * 512:(nt + 1) * 512],
                        start=(k == 0), stop=(k == 7))
                    if k == 7:
                        nc.vector.tensor_copy(
                            qT[1][:, nt * 512:(nt + 1) * 512], q1tile[0][:])

                def stage_a(t):
                    p, qb = t // M, t % M
                    qs = slice(qb * 64, (qb + 1) * 64)
                    fs = fsps.tile([128, 320], F32, tag="fs")
                    sps = fs[:, 0:192]
                    # one 192-wide scores MM per half (s1|s2|diag gathered)
                    nc.tensor.matmul(
                        sps[0:64, :], lhsT=qT[p][0:64, qs],
                        rhs=ksel[p][0:64, 3 * qb:3 * qb + 3, :],
                        start=True, stop=False, skip_group_check=True)
                    nc.tensor.matmul(
                        sps[64:128, :], lhsT=qT[p][64:128, qs],
                        rhs=ksel[p][64:128, 3 * qb:3 * qb + 3, :],
                        start=True, stop=False, skip_group_check=True,
                        tile_position=(64, 64))
                    # dup-diag mask: outer-product accumulate of -30000 flags
                    nc.tensor.matmul(
                        sps[:, 128:192], lhsT=wbm[0:1, p * 32 + qb, :],
                        rhs=ones[0:1, :],
                        start=False, stop=True, skip_group_check=True)
                    pu = wrk.tile([128, 192], F16, tag="pu")
                    den = wrk.tile([128, 1], F32, tag="den")
                    nc.scalar.activation(pu[:], sps[:],
                                         mybir.ActivationFunctionType.Exp,
                                         accum_out=den[:])
                    rden = wrk.tile([128, 1], F32, tag="rden")
                    nc.vector.reciprocal(rden[:], den[:])
                    st[t] = {"fs": fs, "pu": pu, "rden": rden}

                def stage_b(t):
                    s_ = st[t]
                    hs = hsps.tile([128, 448], F16, tag="hs")
                    pt = hs[0:64, 0:384]
                    for s in range(3):
                        nc.tensor.transpose(
                            pt[:, s * 128:(s + 1) * 128],
                            s_["pu"][:, s * 64:(s + 1) * 64], ident[:])
                    pts = wrk.tile([64, 384], F16, tag="pts")
                    nc.vector.tensor_copy(pts[:], pt[:])
                    s_["hs"] = hs
                    s_["pts"] = pts

                def stage_c(t):
                    p, qb = t // M, t % M
                    s_ = st[t]
                    pts = s_["pts"]
                    avpAB = s_["fs"][0:64, 192:320]
                    for h, cs in ((0, slice(0, 64)), (1, slice(64, 128))):
                        off = h * 64
                        for s in range(2):
                            nc.tensor.matmul(
                                avpAB[:, cs],
                                lhsT=pts[:, s * 128 + off:s * 128 + off + 64],
                                rhs=vsel[p][0:64, h, 2 * qb + s, :],
                                start=(s == 0), stop=False)
                        nc.tensor.matmul(
                            avpAB[:, cs],
                            lhsT=pts[:, 256 + off:256 + off + 64],
                            rhs=vdAB[p][:, h, qb * 64:(qb + 1) * 64],
                            start=False, stop=True)
                    # normalized evacuation (scale = 1/den per partition)
                    av_sb = wrk.tile([64, 128], F16, tag="av_sb")
                    nc.scalar.activation(av_sb[:, 0:64], avpAB[:, 0:64],
                                         mybir.ActivationFunctionType.Identity,
                                         scale=s_["rden"][0:64, 0:1])
                    nc.scalar.activation(av_sb[:, 64:128], avpAB[:, 64:128],
                                         mybir.ActivationFunctionType.Identity,
                                         scale=s_["rden"][64:128, 0:1])
                    s_["av_sb"] = av_sb

                def stage_d(t):
                    p, qb = t // M, t % M
                    qs = slice(qb * 64, (qb + 1) * 64)
                    s_ = st.pop(t)
                    otp = s_["hs"][:, 384:448]
                    nc.tensor.transpose(otp[:], s_["av_sb"][:],
                                        ident[0:64, 0:64])
                    nc.vector.tensor_copy(outT[p][:, qs], otp[:])

                if True:
                    def proj_half(tt, nt):
                        ts_ = slice(tt * 128, (tt + 1) * 128)
                        ns = slice(nt * 512, (nt + 1) * 512)
                        yp = qq.tile([128, 512], F32, tag="q")
                        nc.tensor.matmul(yp[:], lhsT=outT[0][:, ts_],
                                         rhs=pwt[:, 0, ns],
                                         start=True, stop=False)
                        nc.tensor.matmul(yp[:], lhsT=outT[1][:, ts_],
                                         rhs=pwt[:, 1, ns],
                                         start=False, stop=True)
                        ys = wrk.tile([128, 512], F16, tag="ys")
                        nc.vector.tensor_copy(ys[:], yp[:])
                        nc.sync.dma_start(y_d[ts_, ns], ys[:])

                    for w in range(NT + 3):
                        if w < NT:
                            stage_a(w)
                            if w < M:
                                q1_mm(w)      # Q1 fills attention p=0
                        if 0 <= w - 1 < NT:
                            stage_b(w - 1)
                        if 0 <= w - 2 < NT:
                            stage_c(w - 2)
                        if 0 <= w - 3 < NT:
                            td = w - 3
                            stage_d(td)
                            # proj fills attention p=1: after stage_d of
                            # (p=1, qb=2tt+1) both pairs of token tile tt
                            # are complete.
                            if td >= M + 1 and (td - M) % 2 == 1:
                                tt = (td - M - 1) // 2
                                proj_half(tt, 0)
                            elif td >= M + 2 and (td - M) % 2 == 0:
                                tt = (td - M - 2) // 2
                                proj_half(tt, 1)
                    proj_half(M // 2 - 1, 1)

    nc.finalize()
    return nc


def _wrap_idx(lists):
    """lists: per 16-partition group g, the unwrapped index list.
    Returns wrapped [16*len(lists), ceil(n/16)] int16 layout."""
    n = len(lists[0])
    cols = (n + 15) // 16
    out = np.zeros((16 * len(lists), cols), np.int16)
    for g, lst in enumerate(lists):
        for j, v in enumerate(lst):
            out[g * 16 + j % 16, j // 16] = v
    return out


def _host_prep(x, qkv_w, proj_w):
    """Per-core input maps + block selection (float64, matches fp32 ref)."""
    in_maps = []
    x64 = x.astype(np.float64)
    for core in range(NCORES):
        b = core // (NCORES // B)
        hg = core % (NCORES // B)
        heads = [hg * HPC + i for i in range(HPC)]

        xt = np.ascontiguousarray(x[b].T).astype(np.float16)

        wqkvT = np.empty((DIM, 768), np.float32)
        for p in range(2):
            hA, hB = heads[2 * p], heads[2 * p + 1]
            wqkvT[:, p*128:p*128+64] = qkv_w[hA*64:(hA+1)*64].T * SCALE
            wqkvT[:, p*128+64:p*128+128] = qkv_w[hB*64:(hB+1)*64].T * SCALE
            kbase = 256 + p * 128
            wqkvT[:, kbase:kbase+64] = qkv_w[DIM+hA*64:DIM+(hA+1)*64].T
            wqkvT[:, kbase+64:kbase+128] = qkv_w[DIM+hB*64:DIM+(hB+1)*64].T
            vbase = 512 + p * 128
            wqkvT[:, vbase:vbase+64] = qkv_w[2*DIM+hA*64:2*DIM+(hA+1)*64].T
            wqkvT[:, vbase+64:vbase+128] = qkv_w[2*DIM+hB*64:2*DIM+(hB+1)*64].T

        pw = np.ascontiguousarray(
            proj_w[:, heads[0]*64:(heads[-1]+1)*64].T).astype(np.float16)

        # float64 selection (matches fp32 reference ordering w/ margin)
        xb = x64[b].reshape(M, BS, DIM).mean(axis=1)
        sel = {}     # (p, h) -> list of (s1, s2) per qblock
        dup = {}     # (p, h) -> list of bool per qblock
        for p in range(2):
            for hip in range(2):
                h = heads[2 * p + hip]
                qb_ = xb @ qkv_w[h*64:(h+1)*64].T.astype(np.float64)
                kb_ = xb @ qkv_w[DIM+h*64:DIM+(h+1)*64].T.astype(np.float64)
                c = qb_ @ kb_.T
                ss, dd = [], []
                for i in range(M):
                    order = np.argsort(-c[i], kind="stable")
                    i1, i2 = int(order[0]), int(order[1])
                    ss.append((i1, i2))
                    dd.append(i == i1 or i == i2)
                sel[(p, hip)] = ss
                dup[(p, hip)] = dd

        # selidx [128, 28]: K gathers (3/qblock incl diag) cols 0:12,
        # V gathers (2/qblock) cols 12:28
        selidx = np.zeros((128, 28), np.int16)
        for p in range(2):
            lists = []
            for hip in range(2):
                lst = []
                for i in range(M):
                    s1, s2 = sel[(p, hip)][i]
                    lst += [s1, s2, i]
                lists += [lst] * 4       # replicate across 4 groups of 16
            kw = _wrap_idx(lists)        # [128, 6]
            selidx[:, p * 6:(p + 1) * 6] = kw
        for p in range(2):
            for hip in range(2):
                lst = []
                for i in range(M):
                    s1, s2 = sel[(p, hip)][i]
                    lst += [s1, s2]
                vw = _wrap_idx([lst] * 4)   # [64, 4]
                c0 = 12 + (p * 2 + hip) * 4
                selidx[0:64, c0:c0 + 4] = vw

        # wbmask [1, 64, 128] f16: -30000 at (p*32+i, hip*64:(hip+1)*64)
        # when diag duplicates a selected block
        wbmask = np.zeros((64, 128), np.float16)
        for p in range(2):
            for hip in range(2):
                for i in range(M):
                    if dup[(p, hip)][i]:
                        wbmask[p * 32 + i, hip*64:(hip+1)*64] = -30000.0
        in_maps.append({"xt": xt, "wq": wqkvT.astype(np.float16), "pw": pw,
                        "selidx": selidx,
                        "wbmask": wbmask.reshape(1, -1)})
    return in_maps


def kernel(x, qkv_w, proj_w, proj_b):
    global _NC_CACHE, LAST_RESULTS
    x = np.asarray(x, np.float32)
    qkv_w = np.asarray(qkv_w, np.float32)
    proj_w = np.asarray(proj_w, np.float32)
    proj_b = np.asarray(proj_b, np.float32)

    if _NC_CACHE is None:
        _NC_CACHE = build_kernel()
    nc = _NC_CACHE

    in_maps = _host_prep(x, qkv_w, proj_w)
    res = run_bass_kernel_spmd(nc, in_maps, list(range(NCORES)))
    LAST_RESULTS = res

    out = np.zeros((B, N, DIM), np.float32)
    for core in range(NCORES):
        out[core // (NCORES // B)] += res.results[core]["y"].astype(np.float32)
    out += proj_b[None, None, :]
    return out


# revision 18
# speedup vs baseline: 1.1454x; 1.1238x over previous
"""Block-sparse attention (CABAttention) Trainium2 kernel — v4.

Sharding: 8 cores = 2 batches x 4 head-groups (4 heads each).
Per core: qkv projection (fp16 in, fp32 PSUM), top-2+diag block-sparse
attention, output projection (row-parallel partial sums, host-summed).

v4 vs v3: the dynamic block selection moves OFF the tensor engine.
GPSIMD ap_gather pre-gathers the selected K blocks (with diag; one
192-wide score matmul per head-half) and V blocks (without diag) into
packed SBUF buffers using host-provided wrapped index tensors, so every
attention matmul is statically addressed — no PE register loads, no
per-iteration ucode ops. The dup-diag mask lands via a 1-contraction
outer-product matmul accumulated into PSUM instead of an ACT pass, and
softmax normalization happens after AV (scale at PSUM evacuation) so the
probability tensor is never rescaled on the DVE. Phase order keeps the
PE dense for the HAM clock gate: K-projection streams k-outer against
the input DMA, Q1-projection fills attention p=0, output projection
fills attention p=1.
"""
import sys

sys.path.insert(0, "/opt/trn_rl_repo")

import numpy as np

import concourse.bass as bass
import concourse.mybir as mybir
import concourse.tile as tile
from concourse import bacc
from concourse.bass_utils import run_bass_kernel_spmd
from concourse.masks import make_identity

F32 = mybir.dt.float32
F16 = mybir.dt.float16
I16 = mybir.dt.int16
ET = mybir.EngineType

DIM = 1024
H = 16
HD = 64
BS = 64
N = 2048
B = 2
M = N // BS            # 32 blocks
SCALE = HD ** -0.5
NCORES = 8
HPC = H // (NCORES // B)   # 4 heads per core

_NC_CACHE = None
LAST_RESULTS = None


def build_kernel():
    nc = bacc.Bacc(None)
    xt_d = nc.dram_tensor("xt", [DIM, N], F16, kind="ExternalInput")
    wq_d = nc.dram_tensor("wq", [DIM, 768], F16, kind="ExternalInput")
    pw_d = nc.dram_tensor("pw", [256, DIM], F16, kind="ExternalInput")
    idx_d = nc.dram_tensor("selidx", [128, 28], I16, kind="ExternalInput")
    wbm_d = nc.dram_tensor("wbmask", [1, 64 * 128], F16, kind="ExternalInput")
    y_d = nc.dram_tensor("y", [N, DIM], F16, kind="ExternalOutput")

    with tile.TileContext(nc) as tc:
        with tc.tile_pool(name="big", bufs=1) as big, \
             tc.tile_pool(name="wrk", bufs=8) as wrk:

            # ---- persistent SBUF tensors ----
            xt = big.tile([128, 8, N], F16)           # x^T, feature-major
            wq = big.tile([128, 8, 768], F16)         # qkv weights^T
            pwt = big.tile([128, 2, DIM], F16)        # proj weights
            idx = big.tile([128, 28], I16)
            wbm = big.tile([1, 64, 128], F16)
            ones = big.tile([1, 64], F16)
            qT = [big.tile([128, N], F16, name=f"qT{i}") for i in range(2)]
            kkT = [big.tile([128, N], F16, name=f"kkT{i}") for i in range(2)]
            vdALL = big.tile([64, 4, N], F16)   # (pair*2+half) major
            ksel = [big.tile([128, 3 * M, 64], F16, name=f"ksel{i}")
                    for i in range(2)]
            vsel = [big.tile([64, 2, 2 * M, 64], F16, name=f"vsel{i}")
                    for i in range(2)]
            outT = [big.tile([128, N], F16, name=f"outT{i}") for i in range(2)]
            identf = big.tile([128, 128], F32)
            ident = big.tile([128, 128], F16)

            # ---- input DMAs (chunked so K-proj streams behind them) ----
            xt_v = xt_d[:].rearrange("(a p) n -> p a n", p=128)
            wq_v = wq_d[:].rearrange("(a p) n -> p a n", p=128)
            pw_v = pw_d[:].rearrange("(a p) n -> p a n", p=128)
            for k in range(8):
                nc.sync.dma_start(wq[:, k, :], wq_v[:, k, :])
                nc.sync.dma_start(xt[:, k, :], xt_v[:, k, :])
            nc.sync.dma_start(pwt[:], pw_v[:])
            nc.sync.dma_start(idx[:], idx_d[:])
            nc.sync.dma_start(wbm[:], wbm_d[:].rearrange("o (c p) -> o c p",
                                                         c=64))

            make_identity(nc, identf[:])
            nc.vector.tensor_copy(ident[:], identf[:])
            nc.gpsimd.memset(ones[:], 1.0)

            # ---- K projection: k-outer, streams behind the input DMA ----
            # wq columns: q0 0:128 | q1 128:256 | k0 256:384 | k1 384:512 |
            #             v (p0 hA|hB, p1 hA|hB) 512:768
            with tc.tile_pool(name="kps", bufs=1, space="PSUM") as kps:
                kp = [[kps.tile([128, 512], F32, name=f"kp{p}{nt}")
                       for nt in range(4)] for p in range(2)]
                for k in range(8):
                    for p in range(2):
                        for nt in range(4):
                            nc.tensor.matmul(
                                kp[p][nt][:],
                                lhsT=wq[:, k, 256 + p * 128:384 + p * 128],
                                rhs=xt[:, k, nt * 512:(nt + 1) * 512],
                                start=(k == 0), stop=(k == 7))
                # evacuations spread across engines
                for p in range(2):
                    for nt in range(4):
                        if nt % 2:
                            nc.scalar.copy(
                                kkT[p][:, nt * 512:(nt + 1) * 512],
                                kp[p][nt][:])
                        else:
                            nc.vector.tensor_copy(
                                kkT[p][:, nt * 512:(nt + 1) * 512],
                                kp[p][nt][:])

            # ---- K gathers (gpsimd; 3 blocks/qblock incl diag) ----
            for p in range(2):
                nc.gpsimd.ap_gather(
                    ksel[p][:].rearrange("p m d -> p (m d)"),
                    kkT[p][:],
                    idx[:, p * 6:(p + 1) * 6],
                    channels=128, num_elems=M, d=64, num_idxs=3 * M)

            # ---- V projection (dense) ----
            with tc.tile_pool(name="vps", bufs=2, space="PSUM") as vps:
                for tt in range(16):
                    ps = vps.tile([128, 256], F32, tag="vps")
                    for k in range(8):
                        nc.tensor.matmul(
                            ps[:], lhsT=xt[:, k, tt * 128:(tt + 1) * 128],
                            rhs=wq[:, k, 512:768],
                            start=(k == 0), stop=(k == 7))
                    # one wide evacuation per half-tile into the packed
                    # (pair, half)-major V layout
                    for half in range(2):
                        j = 2 * tt + half
                        rs = slice(half * 64, (half + 1) * 64)
                        js = slice(j * 64, (j + 1) * 64)
                        nc.vector.tensor_copy(
                            vdALL[:, :, js],
                            ps[rs, :].rearrange("p (g d) -> p g d", g=4))

            # ---- V gathers (gpsimd; 2 blocks/qblock, diag stays static) ----
            for p in range(2):
                for h in range(2):
                    nc.gpsimd.ap_gather(
                        vsel[p][:, h].rearrange("p m d -> p (m d)"),
                        vdALL[:, p * 2 + h, :],
                        idx[0:64, 12 + (p * 2 + h) * 4:16 + (p * 2 + h) * 4],
                        channels=64, num_elems=M, d=64, num_idxs=2 * M)

            # ---- Q0 projection (dense) ----
            # qq pool (2 banks) is shared by Q0, Q1 and the output
            # projection — they are temporally disjoint.
            with tc.tile_pool(name="qq", bufs=2, space="PSUM") as qq, \
                 tc.tile_pool(name="fsps", bufs=3, space="PSUM") as fsps, \
                 tc.tile_pool(name="hsps", bufs=3, space="PSUM") as hsps:

                for nt in range(4):
                    ps = qq.tile([128, 512], F32, tag="q")
                    for k in range(8):
                        nc.tensor.matmul(
                            ps[:], lhsT=wq[:, k, 0:128],
                            rhs=xt[:, k, nt * 512:(nt + 1) * 512],
                            start=(k == 0), stop=(k == 7))
                    if nt % 2:
                        nc.scalar.copy(
                            qT[0][:, nt * 512:(nt + 1) * 512], ps[:])
                    else:
                        nc.vector.tensor_copy(
                            qT[0][:, nt * 512:(nt + 1) * 512], ps[:])

                # ---- Q1 projection (dense) ----
                for nt in range(4):
                    ps = qq.tile([128, 512], F32, tag="q", name="q1t")
                    for k in range(8):
                        nc.tensor.matmul(
                            ps[:], lhsT=wq[:, k, 128:256],
                            rhs=xt[:, k, nt * 512:(nt + 1) * 512],
                            start=(k == 0), stop=(k == 7))
                    if nt % 2:
                        nc.scalar.copy(
                            qT[1][:, nt * 512:(nt + 1) * 512], ps[:])
                    else:
                        nc.vector.tensor_copy(
                            qT[1][:, nt * 512:(nt + 1) * 512], ps[:])

                # ---- attention (qb-major) + interleaved projection ----
                st = {}
                NT = 2 * M

                def stage_a(t):
                    qb, p = t // 2, t % 2
                    qs = slice(qb * 64, (qb + 1) * 64)
                    fs = fsps.tile([128, 256], F32, tag="fs")
                    sps = fs[:, 0:192]
                    # one 192-wide scores MM per half (s1|s2|diag gathered)
                    nc.tensor.matmul(
                        sps[0:64, :], lhsT=qT[p][0:64, qs],
                        rhs=ksel[p][0:64, 3 * qb:3 * qb + 3, :],
                        start=True, stop=False, skip_group_check=True)
                    nc.tensor.matmul(
                        sps[64:128, :], lhsT=qT[p][64:128, qs],
                        rhs=ksel[p][64:128, 3 * qb:3 * qb + 3, :],
                        start=True, stop=False, skip_group_check=True,
                        tile_position=(64, 64))
                    # dup-diag mask: outer-product accumulate of -30000 flags
                    nc.tensor.matmul(
                        sps[:, 128:192], lhsT=wbm[0:1, p * 32 + qb, :],
                        rhs=ones[0:1, :],
                        start=False, stop=True, skip_group_check=True)
                    pu = wrk.tile([128, 192], F16, tag="pu")
                    nc.scalar.activation(pu[:], sps[:],
                                         mybir.ActivationFunctionType.Exp)
                    den = wrk.tile([128, 1], F32, tag="den")
                    nc.vector.reduce_sum(den[:], pu[:],
                                         axis=mybir.AxisListType.X)
                    rden = wrk.tile([128, 1], F32, tag="rden")
                    nc.vector.reciprocal(rden[:], den[:])
                    st[t] = {"fs": fs, "pu": pu, "rden": rden}

                def stage_b(t):
                    s_ = st[t]
                    hs = hsps.tile([128, 448], F16, tag="hs")
                    pt = hs[0:64, 0:384]
                    for s in range(3):
                        nc.tensor.transpose(
                            pt[:, s * 128:(s + 1) * 128],
                            s_["pu"][:, s * 64:(s + 1) * 64], ident[:])
                    pts = wrk.tile([64, 384], F16, tag="pts")
                    nc.vector.tensor_copy(pts[:], pt[:])
                    s_["hs"] = hs
                    s_["pts"] = pts

                def stage_c(t):
                    qb, p = t // 2, t % 2
                    s_ = st[t]
                    pts = s_["pts"]
                    # A-head AV into PSUM partitions 0:64, B-head into
                    # 64:128 (tile col 64) so one scaled evacuation covers
                    # both with the [128,1] rden.
                    for h in range(2):
                        avp = s_["fs"][h * 64:(h + 1) * 64, 192:256]
                        off = h * 64
                        tp = None if h == 0 else (0, 64)
                        for s in range(2):
                            nc.tensor.matmul(
                                avp[:],
                                lhsT=pts[:, s * 128 + off:s * 128 + off + 64],
                                rhs=vsel[p][0:64, h, 2 * qb + s, :],
                                start=(s == 0), stop=False,
                                skip_group_check=True, tile_position=tp)
                        nc.tensor.matmul(
                            avp[:],
                            lhsT=pts[:, 256 + off:256 + off + 64],
                            rhs=vdALL[:, p * 2 + h, qb * 64:(qb + 1) * 64],
                            start=False, stop=True,
                            skip_group_check=True, tile_position=tp)
                    # normalized evacuation (scale = 1/den per partition)
                    av_sb = wrk.tile([128, 64], F16, tag="av_sb")
                    nc.scalar.activation(av_sb[:], s_["fs"][:, 192:256],
                                         mybir.ActivationFunctionType.Identity,
                                         scale=s_["rden"][:, 0:1])
                    s_["av_sb"] = av_sb

                def stage_d(t):
                    qb, p = t // 2, t % 2
                    qs = slice(qb * 64, (qb + 1) * 64)
                    s_ = st.pop(t)
                    otp = s_["hs"][:, 384:448]
                    nc.tensor.transpose(otp[0:64, :], s_["av_sb"][0:64, :],
                                        ident[0:64, 0:64])
                    nc.tensor.transpose(otp[64:128, :],
                                        s_["av_sb"][64:128, :],
                                        ident[64:128, 64:128],
                                        tile_position=(64, 64))
                    nc.vector.tensor_copy(outT[p][:, qs], otp[:])

                if True:
                    def proj_half(tt, nt):
                        ts_ = slice(tt * 128, (tt + 1) * 128)
                        ns = slice(nt * 512, (nt + 1) * 512)
                        yp = qq.tile([128, 512], F32, tag="q")
                        nc.tensor.matmul(yp[:], lhsT=outT[0][:, ts_],
                                         rhs=pwt[:, 0, ns],
                                         start=True, stop=False)
                        nc.tensor.matmul(yp[:], lhsT=outT[1][:, ts_],
                                         rhs=pwt[:, 1, ns],
                                         start=False, stop=True)
                        ys = wrk.tile([128, 512], F16, tag="ys")
                        if nt:
                            nc.scalar.copy(ys[:], yp[:])
                        else:
                            nc.vector.tensor_copy(ys[:], yp[:])
                        nc.sync.dma_start(y_d[ts_, ns], ys[:])

                    # proj after td = 4*tt+3 completes stage D (both pairs
                    # of token tile tt done) — uniform ~1 MM/iter filler.
                    for w in range(NT + 3):
                        if w < NT:
                            stage_a(w)
                        if 0 <= w - 1 < NT:
                            stage_b(w - 1)
                        if 0 <= w - 2 < NT:
                            stage_c(w - 2)
                        if 0 <= w - 3 < NT:
                            td = w - 3
                            stage_d(td)
                            if td % 4 == 3:
                                proj_half(td // 4, 0)
                            elif td % 4 == 1 and td >= 5:
                                proj_half((td - 5) // 4, 1)
                    proj_half(M // 2 - 1, 1)

    nc.finalize()
    return nc


def _wrap_idx(lists):
    """lists: per 16-partition group g, the unwrapped index list.
    Returns wrapped [16*len(lists), ceil(n/16)] int16 layout."""
    n = len(lists[0])
    cols = (n + 15) // 16
    out = np.zeros((16 * len(lists), cols), np.int16)
    for g, lst in enumerate(lists):
        for j, v in enumerate(lst):
            out[g * 16 + j % 16, j // 16] = v
    return out


def _host_prep(x, qkv_w, proj_w):
    """Per-core input maps + block selection (float64, matches fp32 ref)."""
    in_maps = []
    x64 = x.astype(np.float64)
    for core in range(NCORES):
        b = core // (NCORES // B)
        hg = core % (NCORES // B)
        heads = [hg * HPC + i for i in range(HPC)]

        xt = np.ascontiguousarray(x[b].T).astype(np.float16)

        wqkvT = np.empty((DIM, 768), np.float32)
        for p in range(2):
            hA, hB = heads[2 * p], heads[2 * p + 1]
            wqkvT[:, p*128:p*128+64] = qkv_w[hA*64:(hA+1)*64].T * SCALE
            wqkvT[:, p*128+64:p*128+128] = qkv_w[hB*64:(hB+1)*64].T * SCALE
            kbase = 256 + p * 128
            wqkvT[:, kbase:kbase+64] = qkv_w[DIM+hA*64:DIM+(hA+1)*64].T
            wqkvT[:, kbase+64:kbase+128] = qkv_w[DIM+hB*64:DIM+(hB+1)*64].T
            vbase = 512 + p * 128
            wqkvT[:, vbase:vbase+64] = qkv_w[2*DIM+hA*64:2*DIM+(hA+1)*64].T
            wqkvT[:, vbase+64:vbase+128] = qkv_w[2*DIM+hB*64:2*DIM+(hB+1)*64].T

        pw = np.ascontiguousarray(
            proj_w[:, heads[0]*64:(heads[-1]+1)*64].T).astype(np.float16)

        # float64 selection (matches fp32 reference ordering w/ margin)
        xb = x64[b].reshape(M, BS, DIM).mean(axis=1)
        sel = {}     # (p, h) -> list of (s1, s2) per qblock
        dup = {}     # (p, h) -> list of bool per qblock
        for p in range(2):
            for hip in range(2):
                h = heads[2 * p + hip]
                qb_ = xb @ qkv_w[h*64:(h+1)*64].T.astype(np.float64)
                kb_ = xb @ qkv_w[DIM+h*64:DIM+(h+1)*64].T.astype(np.float64)
                c = qb_ @ kb_.T
                ss, dd = [], []
                for i in range(M):
                    order = np.argsort(-c[i], kind="stable")
                    i1, i2 = int(order[0]), int(order[1])
                    ss.append((i1, i2))
                    dd.append(i == i1 or i == i2)
                sel[(p, hip)] = ss
                dup[(p, hip)] = dd

        # selidx [128, 28]: K gathers (3/qblock incl diag) cols 0:12,
        # V gathers (2/qblock) cols 12:28
        selidx = np.zeros((128, 28), np.int16)
        for p in range(2):
            lists = []
            for hip in range(2):
                lst = []
                for i in range(M):
                    s1, s2 = sel[(p, hip)][i]
                    lst += [s1, s2, i]
                lists += [lst] * 4       # replicate across 4 groups of 16
            kw = _wrap_idx(lists)        # [128, 6]
            selidx[:, p * 6:(p + 1) * 6] = kw
        for p in range(2):
            for hip in range(2):
                lst = []
                for i in range(M):
                    s1, s2 = sel[(p, hip)][i]
                    lst += [s1, s2]
                vw = _wrap_idx([lst] * 4)   # [64, 4]
                c0 = 12 + (p * 2 + hip) * 4
                selidx[0:64, c0:c0 + 4] = vw

        # wbmask [1, 64, 128] f16: -30000 at (p*32+i, hip*64:(hip+1)*64)
        # when diag duplicates a selected block
        wbmask = np.zeros((64, 128), np.float16)
        for p in range(2):
            for hip in range(2):
                for i in range(M):
                    if dup[(p, hip)][i]:
                        wbmask[p * 32 + i, hip*64:(hip+1)*64] = -30000.0
        in_maps.append({"xt": xt, "wq": wqkvT.astype(np.float16), "pw": pw,
                        "selidx": selidx,
                        "wbmask": wbmask.reshape(1, -1)})
    return in_maps


def kernel(x, qkv_w, proj_w, proj_b):
    global _NC_CACHE, LAST_RESULTS
    x = np.asarray(x, np.float32)
    qkv_w = np.asarray(qkv_w, np.float32)
    proj_w = np.asarray(proj_w, np.float32)
    proj_b = np.asarray(proj_b, np.float32)

    if _NC_CACHE is None:
        _NC_CACHE = build_kernel()
    nc = _NC_CACHE

    in_maps = _host_prep(x, qkv_w, proj_w)
    res = run_bass_kernel_spmd(nc, in_maps, list(range(NCORES)))
    LAST_RESULTS = res

    out = np.zeros((B, N, DIM), np.float32)
    for core in range(NCORES):
        out[core // (NCORES // B)] += res.results[core]["y"].astype(np.float32)
    out += proj_b[None, None, :]
    return out
